# revision 4
# baseline (speedup 1.0000x reference)
"""Trainium2 Bass kernel for the DGL-style heterogeneous temporal GNN (v2).

Model (per reference):
  for t: h1 = relu(sum_r GraphConv_r(feat[t])); h2 = relu(sum_r GraphConv_r(h1))
  h_last = GRU over t of h2; logits = MLP(h_last)

v2 design (vs. v1 baseline):
 - ship ~25x fewer bytes: feat shards bf16 (device AllGather replicates),
   per-slot gather tables gi (i32 padded src row) + sc (bf16 w/deg).
 - inputs pre-placed with device_put(NamedSharding) so the jitted shard_map
   call doesn't re-transfer unsharded numpy args through the axon relay.
 - bf16 message/aggregate datapath (fp32 accumulation inside DVE reduce and
   PSUM): enables DMA-transpose ([rows,128]bf16 -> [128,rows]) so the dense
   conv phases need no TensorE transposes.
 - agg layout [NSP, 4, 64]: relations r0,r1,r2 + a "z" lane whose feature 0
   is pre-set to 1.0, so conv bias rides in the weight matrix (row 64).
"""
import sys

sys.path.insert(0, "/opt/trn_rl_repo")
import numpy as np
import ml_dtypes

BF16 = ml_dtypes.bfloat16
TRACE = False
LAST_EXEC_NS = None

T, R, N, E, D = 4, 3, 100000, 800000, 64
NC = 8
NS = N // NC            # 12500
NSP = 12800             # padded shard rows
NV2 = NC * NSP          # 102400
KTILE = 256             # max slot columns per gather tile
GCL = (2, 4, 6, 8, 10, 12, 14, 16, 20, 24, 32, 64)
GBIG = 64
DUMMY = NS * 4          # scatter row for padded groups (pad region)
AGG_ROWS = NSP * 4
MCH = 512               # dense-phase chunk (locs)
NCHK = NSP // MCH       # 25


def _segment_arange(sizes):
    total = int(sizes.sum())
    if total == 0:
        return np.zeros(0, np.int64)
    starts = np.cumsum(sizes) - sizes
    return np.arange(total) - np.repeat(starts, sizes)


# --------------------------------------------------------------------------
# host-side preprocessing
# --------------------------------------------------------------------------

def preprocess(src, dst, ew):
    """Per-core edge tables + shared tile descriptors.

    Returns (per_core, tiles): per_core[c] = {gi_t: [128,CK] i32,
    sc_t: [128,CK] bf16, sd_t: [128,CQ] i32}; tiles[t] = list of
    dict(G,k,q,over) shared by all cores (SPMD-uniform).
    """
    src = np.asarray(src, np.int64).reshape(T, R * E)
    dst = np.asarray(dst, np.int64).reshape(T, R * E)
    ew = np.asarray(ew, np.float32).reshape(T, R * E)
    r_of_e = np.repeat(np.arange(R, dtype=np.int64), E)
    tiles = {t: [] for t in range(T)}
    per_core = [dict() for _ in range(NC)]

    for t in range(T):
        st, dt_, w_t = src[t], dst[t], ew[t]
        deg = np.bincount(r_of_e * N + dt_, minlength=R * N)
        wn = w_t / np.maximum(deg[r_of_e * N + dt_], 1).astype(np.float32)
        c = dt_ // NS
        loc = dt_ - c * NS
        key = (c * R + r_of_e) * NSP + loc
        order = np.argsort(key, kind="stable")
        skey = key[order]
        sgi = ((st // NS) * NSP + st % NS)[order].astype(np.int32)
        sw = wn[order].astype(BF16)
        # group boundaries (sorted by core, rel, loc)
        bnd = np.nonzero(np.diff(skey))[0] + 1
        gstart = np.concatenate([[0], bnd]).astype(np.int64)
        gend = np.concatenate([bnd, [skey.size]]).astype(np.int64)
        gcnt = gend - gstart
        gkey = skey[gstart]
        gc = gkey // (R * NSP)
        grem = gkey - gc * (R * NSP)
        gr = grem // NSP
        gloc = grem - gr * NSP
        gdst = gloc * 4 + gr
        # split groups into windows: one main (<=GBIG) + overflow chunks
        nw = 1 + (np.maximum(gcnt - GBIG, 0) + GBIG - 1) // GBIG
        wg = np.repeat(np.arange(gcnt.size), nw)
        wi = _segment_arange(nw)
        wstart = gstart[wg] + wi * GBIG
        wsize = np.minimum(gcnt[wg] - wi * GBIG, GBIG)
        wover = wi > 0
        wc = gc[wg]
        wdst = gdst[wg]
        wcls = np.searchsorted(np.asarray(GCL), wsize)

        gis = [[] for _ in range(NC)]
        scs = [[] for _ in range(NC)]
        sds = [[] for _ in range(NC)]

        def emit(sel_per_core, G, is_over):
            ngmax = max(s.size for s in sel_per_core)
            if ngmax == 0:
                return
            gpt = (128 * KTILE) // G
            ntiles = -(-ngmax // gpt)
            for ti in range(ntiles):
                glo, ghi = ti * gpt, min(ngmax, (ti + 1) * gpt)
                ng_pad = ghi - glo
                k = -(-(ng_pad * G) // 128)
                k = -(-k // G) * G
                q = k // G
                ngt = 128 * q
                tiles[t].append(dict(G=G, k=k, q=q, over=is_over))
                for cc in range(NC):
                    sel = sel_per_core[cc][glo:ghi]
                    giA = np.zeros(128 * k, np.int32)
                    scA = np.zeros(128 * k, BF16)
                    sdA = np.full(ngt, DUMMY, np.int32)
                    sdA[: sel.size] = wdst[sel]
                    sizes = wsize[sel]
                    within = _segment_arange(sizes)
                    slots = np.repeat(np.arange(sel.size) * G, sizes) + within
                    epos = np.repeat(wstart[sel], sizes) + within
                    giA[slots] = sgi[epos]
                    scA[slots] = sw[epos]
                    gis[cc].append(giA.reshape(128, k))
                    scs[cc].append(scA.reshape(128, k))
                    sds[cc].append(sdA.reshape(128, q))

        for ci, G in enumerate(GCL):
            m = (~wover) & (wcls == ci)
            emit([np.nonzero(m & (wc == cc))[0] for cc in range(NC)], G, False)
        emit([np.nonzero(wover & (wc == cc))[0] for cc in range(NC)],
             GBIG, True)

        for cc in range(NC):
            per_core[cc][f"gi_{t}"] = (np.concatenate(gis[cc], 1)
                                       if gis[cc] else np.zeros((128, 0), np.int32))
            per_core[cc][f"sc_{t}"] = (np.concatenate(scs[cc], 1)
                                       if scs[cc] else np.zeros((128, 0), BF16))
            per_core[cc][f"sd_{t}"] = (np.concatenate(sds[cc], 1)
                                       if sds[cc] else np.zeros((128, 0), np.int32))
    return per_core, tiles


def make_feat_shards(feat):
    feat = np.asarray(feat, np.float32).reshape(T, NC, NS, D)
    x = np.zeros((NC, T, NSP, D), BF16)
    for cc in range(NC):
        x[cc, :, :NS, :] = feat[:, cc].astype(BF16)
    return [np.ascontiguousarray(x[cc]) for cc in range(NC)]


def make_weights(W1, b1, W2, b2, Wih, Whh, bih, bhh, Wc1, bc1, Wc2, bc2):
    f = np.float32

    def bf(x):
        return np.ascontiguousarray(np.asarray(x, f)).astype(BF16)

    W1 = np.asarray(W1, f)
    W2 = np.asarray(W2, f)
    b1s = np.asarray(b1, f).sum(0)[None, :]
    b2s = np.asarray(b2, f).sum(0)[None, :]
    z63 = np.zeros((63, D), f)
    Wih = np.asarray(Wih, f)
    Whh = np.asarray(Whh, f)
    bih = np.asarray(bih, f)
    bhh = np.asarray(bhh, f)
    return dict(
        w1s=bf(np.vstack([W1[0], W1[1]])),
        w1r2=bf(np.vstack([W1[2], b1s, z63])),
        w2s=bf(np.vstack([W2[0], W2[1]])),
        w2r2=bf(np.vstack([W2[2], b2s, z63])),
        wihrz=Wih[0:2 * D].T.copy(),
        whhrz=Whh[0:2 * D].T.copy(),
        wihn=Wih[2 * D:3 * D].T.copy(),
        whhn=Whh[2 * D:3 * D].T.copy(),
        gbr=(bih + bhh)[0:D][:, None].copy(),
        gbz=(bih + bhh)[D:2 * D][:, None].copy(),
        gbhn=bhh[2 * D:][:, None].copy(),
        gbin=bih[2 * D:][:, None].copy(),
        wc1=np.asarray(Wc1, f).copy(),
        bc1c=np.asarray(bc1, f)[:, None].copy(),
        wc2=np.asarray(Wc2, f).copy(),
        bc2c=np.asarray(bc2, f).reshape(1, 1).copy(),
    )


W_SHAPES = dict(
    w1s=([2 * D, D], "bf16"), w1r2=([2 * D, D], "bf16"),
    w2s=([2 * D, D], "bf16"), w2r2=([2 * D, D], "bf16"),
    wihrz=([D, 2 * D], "f32"), whhrz=([D, 2 * D], "f32"),
    wihn=([D, D], "f32"), whhn=([D, D], "f32"),
    gbr=([D, 1], "f32"), gbz=([D, 1], "f32"),
    gbhn=([D, 1], "f32"), gbin=([D, 1], "f32"),
    wc1=([D, D], "f32"), bc1c=([D, 1], "f32"),
    wc2=([D, 1], "f32"), bc2c=([1, 1], "f32"),
)


# --------------------------------------------------------------------------
# device program
# --------------------------------------------------------------------------

def build_program(tiles):
    from concourse import bacc, bass, mybir, tile

    f32 = mybir.dt.float32
    i32 = mybir.dt.int32
    bf16 = mybir.dt.bfloat16
    ALU = mybir.AluOpType
    AF = mybir.ActivationFunctionType
    DT = {"f32": f32, "bf16": bf16}

    nc = bacc.Bacc("TRN2", target_bir_lowering=False, debug=False)

    xsh_d = nc.dram_tensor("xsh", [T, NSP, D], bf16, kind="ExternalInput")
    gi_d, sc_d, sd_d = {}, {}, {}
    for t in range(T):
        ck = sum(td["k"] for td in tiles[t])
        cq = sum(td["q"] for td in tiles[t])
        gi_d[t] = nc.dram_tensor(f"gi_{t}", [128, ck], i32, kind="ExternalInput")
        sc_d[t] = nc.dram_tensor(f"sc_{t}", [128, ck], bf16, kind="ExternalInput")
        sd_d[t] = nc.dram_tensor(f"sd_{t}", [128, cq], i32, kind="ExternalInput")
    w_d = {nm: nc.dram_tensor(nm, sh, DT[dt], kind="ExternalInput")
           for nm, (sh, dt) in W_SHAPES.items()}
    out_d = nc.dram_tensor("out", [1, NS], f32, kind="ExternalOutput")

    featf = [nc.dram_tensor(f"featf{t}", [NV2, D], bf16, addr_space="Shared")
             for t in range(T)]
    xin = [nc.dram_tensor(f"xin{t}", [NSP, D], bf16) for t in range(T)]
    agg_d = [nc.dram_tensor(f"agg{i}", [AGG_ROWS, D], bf16) for i in range(2)]
    ag_in = [nc.dram_tensor(f"agin{t}", [NSP, D], bf16) for t in range(T)]
    h1f = [nc.dram_tensor(f"h1f{t}", [NV2, D], bf16, addr_space="Shared")
           for t in range(T)]
    h2T = [nc.dram_tensor(f"h2T{t}", [D, NSP], f32) for t in range(T)]

    with tile.TileContext(nc) as tc:
        with tc.tile_pool(name="const", bufs=1) as cpool:
            wt = {nm: cpool.tile(sh, DT[dt], tag=nm, name="w_" + nm)
                  for nm, (sh, dt) in W_SHAPES.items()}
            for nm in W_SHAPES:
                nc.sync.dma_start(wt[nm][:], w_d[nm][:])
            # zero tile with the z-lane 1.0 marker at feature offset 192
            zt = cpool.tile([128, 25, 256], bf16, tag="zt")
            nc.vector.memset(zt[:], 0.0)
            nc.vector.memset(zt[:, :, 192:193], 1.0)

            def edge_phase(t, src_d, agg, pool):
                for zci in range(4):
                    lo = zci * 3200
                    nc.sync.dma_start(
                        agg[lo * 4:(lo + 3200) * 4].rearrange(
                            "(j p h) d -> p j (h d)", p=128, h=4),
                        zt[:])
                kofs = qofs = 0
                for td in tiles[t]:
                    G, k, q = td["G"], td["k"], td["q"]
                    git = pool.tile([128, k], i32, tag="git")
                    nc.scalar.dma_start(git[:], gi_d[t][:, kofs:kofs + k])
                    sct = pool.tile([128, k], bf16, tag="sct")
                    nc.scalar.dma_start(sct[:], sc_d[t][:, kofs:kofs + k])
                    sdt = pool.tile([128, q], i32, tag="sdt")
                    nc.scalar.dma_start(sdt[:], sd_d[t][:, qofs:qofs + q])
                    msgs = pool.tile([128, k, D], bf16, tag="msgs")
                    for j in range(k):
                        nc.gpsimd.indirect_dma_start(
                            out=msgs[:, j, :], out_offset=None, in_=src_d[:],
                            in_offset=bass.IndirectOffsetOnAxis(
                                ap=git[:, j:j + 1], axis=0))
                    nc.vector.tensor_tensor(
                        out=msgs[:], in0=msgs[:],
                        in1=sct[:, :, None].to_broadcast([128, k, D]),
                        op=ALU.mult)
                    grp = pool.tile([128, q, D], bf16, tag="grp")
                    with nc.allow_low_precision(reason="bf16 group rows"):
                        nc.vector.tensor_reduce(
                            out=grp[:],
                            in_=msgs[:].rearrange("p (q g) d -> p q d g", g=G),
                            axis=mybir.AxisListType.X, op=ALU.add)
                    op = ALU.add if td["over"] else ALU.bypass
                    for jq in range(q):
                        nc.gpsimd.indirect_dma_start(
                            out=agg[:],
                            out_offset=bass.IndirectOffsetOnAxis(
                                ap=sdt[:, jq:jq + 1], axis=0),
                            in_=grp[:, jq, :], in_offset=None,
                            compute_op=op)
                    kofs += k
                    qofs += q

            def load_aT(agg, c0, pool):
                aggv = agg[:].rearrange("(n h) d -> n (h d)", h=4)
                at01 = pool.tile([128, MCH], bf16, tag="at01")
                at2z = pool.tile([128, MCH], bf16, tag="at2z")
                nc.sync.dma_start(at01[:], aggv[c0:c0 + MCH, 0:128],
                                  transpose=True)
                nc.sync.dma_start(at2z[:], aggv[c0:c0 + MCH, 128:256],
                                  transpose=True)
                return at01, at2z

            def dense1(t, agg, pool, psum):
                for m in range(NCHK):
                    c0 = m * MCH
                    at01, at2z = load_aT(agg, c0, pool)
                    po = psum.tile([128, 4, D], f32, tag="po1", space="PSUM")
                    for j in range(4):
                        nc.tensor.matmul(po[:, j, :],
                                         at01[:, j * 128:(j + 1) * 128],
                                         wt["w1s"][:], start=True, stop=False)
                        nc.tensor.matmul(po[:, j, :],
                                         at2z[:, j * 128:(j + 1) * 128],
                                         wt["w1r2"][:], start=False, stop=True)
                    h1t = pool.tile([128, 4, D], bf16, tag="h1t")
                    nc.scalar.activation(h1t[:], po[:], AF.Relu)
                    nc.sync.dma_start(
                        ag_in[t][c0:c0 + MCH].rearrange("(j p) d -> p j d",
                                                        p=128),
                        h1t[:])

            def dense2(t, agg, pool, psum):
                for m in range(NCHK):
                    c0 = m * MCH
                    at01, at2z = load_aT(agg, c0, pool)
                    po2 = psum.tile([D, MCH], f32, tag="po2", space="PSUM")
                    nc.tensor.matmul(po2[:], wt["w2s"][:], at01[:],
                                     start=True, stop=False)
                    nc.tensor.matmul(po2[:], wt["w2r2"][:], at2z[:],
                                     start=False, stop=True)
                    h2t = pool.tile([D, MCH], f32, tag="h2t")
                    nc.scalar.activation(h2t[:], po2[:], AF.Relu)
                    nc.sync.dma_start(h2T[t][:, c0:c0 + MCH], h2t[:])

            with (
                tc.tile_pool(name="work", bufs=2) as pool,
                tc.tile_pool(name="ps", bufs=2, space="PSUM") as psum,
            ):
                for t in range(T):
                    # collectives cannot read IO tensors: stage via SBUF
                    xst = pool.tile([128, NSP // 128, D], bf16, tag="xst")
                    nc.sync.dma_start(
                        xst[:], xsh_d[t].rearrange("(j p) d -> p j d", p=128))
                    nc.sync.dma_start(
                        xin[t][:].rearrange("(j p) d -> p j d", p=128), xst[:])
                for t in range(T):
                    nc.gpsimd.collective_compute(
                        "AllGather", ALU.bypass,
                        replica_groups=[list(range(NC))],
                        ins=[xin[t][:]], outs=[featf[t][:]])
                for t in range(T):
                    agg = agg_d[t % 2]
                    edge_phase(t, featf[t], agg, pool)
                    dense1(t, agg, pool, psum)
                    nc.gpsimd.collective_compute(
                        "AllGather", ALU.bypass,
                        replica_groups=[list(range(NC))],
                        ins=[ag_in[t][:]], outs=[h1f[t][:]])
                for t in range(T):
                    agg = agg_d[t % 2]
                    edge_phase(t, h1f[t], agg, pool)
                    dense2(t, agg, pool, psum)

            # ---- GRU + MLP over feature-major chunks
            with (
                tc.tile_pool(name="gw", bufs=2) as gpool,
                tc.tile_pool(name="gp", bufs=1, space="PSUM") as gps,
            ):
                lrow = gpool.tile([1, NSP], f32, tag="lrow")
                for m in range(NCHK):
                    cols = slice(m * MCH, (m + 1) * MCH)
                    hA = gpool.tile([D, MCH], f32, tag="hA")
                    hB = gpool.tile([D, MCH], f32, tag="hB")
                    nc.vector.memset(hA[:], 0.0)
                    for t in range(T):
                        hin = hA if t % 2 == 0 else hB
                        hout = hB if t % 2 == 0 else hA
                        xT = gpool.tile([D, MCH], f32, tag="xT")
                        nc.sync.dma_start(xT[:], h2T[t][:, cols])
                        ps_r = gps.tile([D, MCH], f32, tag="ps_r", space="PSUM")
                        nc.tensor.matmul(ps_r[:], wt["wihrz"][:, 0:D], xT[:],
                                         start=True, stop=False)
                        nc.tensor.matmul(ps_r[:], wt["whhrz"][:, 0:D], hin[:],
                                         start=False, stop=True)
                        ps_z = gps.tile([D, MCH], f32, tag="ps_z", space="PSUM")
                        nc.tensor.matmul(ps_z[:], wt["wihrz"][:, D:2 * D],
                                         xT[:], start=True, stop=False)
                        nc.tensor.matmul(ps_z[:], wt["whhrz"][:, D:2 * D],
                                         hin[:], start=False, stop=True)
                        ps_n = gps.tile([D, MCH], f32, tag="ps_n", space="PSUM")
                        nc.tensor.matmul(ps_n[:], wt["wihn"][:], xT[:],
                                         start=True, stop=True)
                        ps_h = gps.tile([D, MCH], f32, tag="ps_h", space="PSUM")
                        nc.tensor.matmul(ps_h[:], wt["whhn"][:], hin[:],
                                         start=True, stop=True)
                        r_sb = gpool.tile([D, MCH], f32, tag="r_sb")
                        nc.scalar.activation(r_sb[:], ps_r[:], AF.Sigmoid,
                                             bias=wt["gbr"][:])
                        z_sb = gpool.tile([D, MCH], f32, tag="z_sb")
                        nc.scalar.activation(z_sb[:], ps_z[:], AF.Sigmoid,
                                             bias=wt["gbz"][:])
                        hn = gpool.tile([D, MCH], f32, tag="hn")
                        nc.scalar.activation(hn[:], ps_h[:], AF.Identity,
                                             bias=wt["gbhn"][:])
                        nc.vector.tensor_tensor(out=hn[:], in0=r_sb[:],
                                                in1=hn[:], op=ALU.mult)
                        nc.vector.tensor_tensor(out=hn[:], in0=ps_n[:],
                                                in1=hn[:], op=ALU.add)
                        nt = gpool.tile([D, MCH], f32, tag="nt")
                        nc.scalar.activation(nt[:], hn[:], AF.Tanh,
                                             bias=wt["gbin"][:])
                        nc.vector.tensor_tensor(out=hout[:], in0=hin[:],
                                                in1=nt[:], op=ALU.subtract)
                        nc.vector.tensor_tensor(out=hout[:], in0=z_sb[:],
                                                in1=hout[:], op=ALU.mult)
                        nc.vector.tensor_tensor(out=hout[:], in0=nt[:],
                                                in1=hout[:], op=ALU.add)
                    hlast = hA if T % 2 == 0 else hB
                    ps_f = gps.tile([D, MCH], f32, tag="ps_f", space="PSUM")
                    nc.tensor.matmul(ps_f[:], wt["wc1"][:], hlast[:],
                                     start=True, stop=True)
                    zf = gpool.tile([D, MCH], f32, tag="zf")
                    nc.scalar.activation(zf[:], ps_f[:], AF.Relu,
                                         bias=wt["bc1c"][:])
                    ps_l = gps.tile([1, MCH], f32, tag="ps_l", space="PSUM")
                    nc.tensor.matmul(ps_l[:], wt["wc2"][:], zf[:],
                                     start=True, stop=True)
                    nc.scalar.activation(lrow[:, cols], ps_l[:], AF.Identity,
                                         bias=wt["bc2c"][:])
                nc.sync.dma_start(out_d[:], lrow[:, 0:NS])

    nc.compile()
    return nc


# --------------------------------------------------------------------------
# fast PJRT runner: pre-place sharded inputs with device_put so the jitted
# shard_map call doesn't re-transfer unsharded numpy args through the relay
# --------------------------------------------------------------------------

def _install_fast_pjrt_runner():
    import jax
    from jax.sharding import Mesh, PartitionSpec, NamedSharding
    from jax.experimental.shard_map import shard_map
    from concourse import bass2jax, mybir
    from concourse.bass2jax import (_bass_exec_p, install_neuronx_cc_hook,
                                    partition_id_tensor)

    def run_fast(nc, in_maps, n_cores):
        install_neuronx_cc_hook()
        partition_name = (nc.partition_id_tensor.name
                          if nc.partition_id_tensor else None)
        in_names, out_names, out_avals, zero_outs = [], [], [], []
        for alloc in nc.m.functions[0].allocations:
            if not isinstance(alloc, mybir.MemoryLocationSet):
                continue
            name = alloc.memorylocations[0].name
            if alloc.kind == "ExternalInput":
                if name != partition_name:
                    in_names.append(name)
            elif alloc.kind == "ExternalOutput":
                out_names.append(name)
                shape = tuple(alloc.tensor_shape)
                dtype = mybir.dt.np(alloc.dtype)
                out_avals.append(jax.core.ShapedArray(shape, dtype))
                zero_outs.append(np.zeros(shape, dtype))
        n_params = len(in_names)
        in_names.extend(out_names)
        if partition_name is not None:
            in_names.append(partition_name)

        def _body(*args):
            operands = list(args)
            if partition_name is not None:
                operands.append(partition_id_tensor())
            outs = _bass_exec_p.bind(
                *operands, out_avals=tuple(out_avals),
                in_names=tuple(in_names), out_names=tuple(out_names),
                lowering_input_output_aliases=(),
                sim_require_finite=True, sim_require_nnan=True, nc=nc)
            return tuple(outs)

        devices = jax.devices()[:n_cores]
        mesh = Mesh(np.asarray(devices), ("core",))
        n_outs = len(out_avals)
        in_specs = (PartitionSpec("core"),) * (n_params + n_outs)
        out_specs = (PartitionSpec("core"),) * len(out_names)
        sharded = jax.jit(
            shard_map(_body, mesh=mesh, in_specs=in_specs,
                      out_specs=out_specs, check_rep=False),
            keep_unused=True)
        concat_in = [
            np.concatenate([np.asarray(m[name]) for m in in_maps], axis=0)
            for name in in_names[:n_params]]
        concat_zeros = [
            np.zeros((n_cores * z.shape[0], *z.shape[1:]), z.dtype)
            for z in zero_outs]
        sh = NamedSharding(mesh, PartitionSpec("core"))
        dev_in = [jax.device_put(a, sh) for a in concat_in]
        dev_zero = [jax.device_put(z, sh) for z in concat_zeros]
        out_arrs = sharded(*dev_in, *dev_zero)
        return [
            {name: np.asarray(out_arrs[i]).reshape(
                n_cores, *out_avals[i].shape)[c]
             for i, name in enumerate(out_names)}
            for c in range(n_cores)
        ]

    bass2jax.run_bass_via_pjrt = run_fast


# --------------------------------------------------------------------------
# entry point
# --------------------------------------------------------------------------

def kernel(**inputs):
    import time

    per_core, tiles = preprocess(inputs["src"], inputs["dst"], inputs["ew"])
    wts = make_weights(
        inputs["W1"], inputs["b1"], inputs["W2"], inputs["b2"],
        inputs["Wih"], inputs["Whh"], inputs["bih"], inputs["bhh"],
        inputs["Wc1"], inputs["bc1"], inputs["Wc2"], inputs["bc2"])
    shards = make_feat_shards(inputs["feat"])
    nc = build_program(tiles)

    in_maps = []
    for cc in range(NC):
        m = dict(per_core[cc])
        m.update(wts)
        m["xsh"] = shards[cc]
        in_maps.append(m)

    _install_fast_pjrt_runner()
    from concourse.bass_utils import run_bass_kernel_spmd
    kwargs = {}
    if TRACE:
        kwargs = dict(trace=True, trace_cores=list(range(NC)))
    t0 = time.monotonic()
    try:
        res = run_bass_kernel_spmd(nc, in_maps, list(range(NC)), **kwargs)
    except (ImportError, ModuleNotFoundError):
        # NTFF profiling hook unavailable in this environment
        res = run_bass_kernel_spmd(nc, in_maps, list(range(NC)))
    wall_ns = (time.monotonic() - t0) * 1e9
    global LAST_EXEC_NS
    LAST_EXEC_NS = res.exec_time_ns if res.exec_time_ns else int(wall_ns)
    out = np.concatenate(
        [np.asarray(res.results[cc]["out"]).reshape(NS) for cc in range(NC)])
    return out.astype(np.float32)


if __name__ == "__main__":
    pass


# revision 5
# speedup vs baseline: 13.1061x; 13.1061x over previous
"""Trainium2 Bass kernel for the DGL-style heterogeneous temporal GNN (v2).

Model (per reference):
  for t: h1 = relu(sum_r GraphConv_r(feat[t])); h2 = relu(sum_r GraphConv_r(h1))
  h_last = GRU over t of h2; logits = MLP(h_last)

v2 design (vs. v1 baseline):
 - ship ~25x fewer bytes: feat shards bf16 (device AllGather replicates),
   per-slot gather tables gi (i32 padded src row) + sc (bf16 w/deg).
 - inputs pre-placed with device_put(NamedSharding) so the jitted shard_map
   call doesn't re-transfer unsharded numpy args through the axon relay.
 - bf16 message/aggregate datapath (fp32 accumulation inside DVE reduce and
   PSUM): enables DMA-transpose ([rows,128]bf16 -> [128,rows]) so the dense
   conv phases need no TensorE transposes.
 - agg layout [NSP, 4, 64]: relations r0,r1,r2 + a "z" lane whose feature 0
   is pre-set to 1.0, so conv bias rides in the weight matrix (row 64).
"""
import sys

sys.path.insert(0, "/opt/trn_rl_repo")
import numpy as np
import ml_dtypes

BF16 = ml_dtypes.bfloat16
TRACE = False
LAST_EXEC_NS = None

T, R, N, E, D = 4, 3, 100000, 800000, 64
NC = 8
NS = N // NC            # 12500
NSP = 12800             # padded shard rows
NV2 = NC * NSP          # 102400
KTILE = 256             # max slot columns per gather tile
GCL = (2, 4, 6, 8, 10, 12, 14, 16, 20, 24, 32, 64)
GBIG = 64
DUMMY = NS * 4          # scatter row for padded groups (pad region)
AGG_ROWS = NSP * 4
MCH = 512               # dense-phase chunk (locs)
NCHK = NSP // MCH       # 25


def _segment_arange(sizes):
    total = int(sizes.sum())
    if total == 0:
        return np.zeros(0, np.int64)
    starts = np.cumsum(sizes) - sizes
    return np.arange(total) - np.repeat(starts, sizes)


# --------------------------------------------------------------------------
# host-side preprocessing
# --------------------------------------------------------------------------

def preprocess(src, dst, ew):
    """Per-core edge tables + shared tile descriptors.

    Returns (per_core, tiles): per_core[c] = {gi_t: [128,CK] i32,
    sc_t: [128,CK] bf16, sd_t: [128,CQ] i32}; tiles[t] = list of
    dict(G,k,q,over) shared by all cores (SPMD-uniform).
    """
    src = np.asarray(src, np.int64).reshape(T, R * E)
    dst = np.asarray(dst, np.int64).reshape(T, R * E)
    ew = np.asarray(ew, np.float32).reshape(T, R * E)
    r_of_e = np.repeat(np.arange(R, dtype=np.int64), E)
    tiles = {t: [] for t in range(T)}
    per_core = [dict() for _ in range(NC)]

    for t in range(T):
        st, dt_, w_t = src[t], dst[t], ew[t]
        deg = np.bincount(r_of_e * N + dt_, minlength=R * N)
        wn = w_t / np.maximum(deg[r_of_e * N + dt_], 1).astype(np.float32)
        c = dt_ // NS
        loc = dt_ - c * NS
        key = (c * R + r_of_e) * NSP + loc
        order = np.argsort(key, kind="stable")
        skey = key[order]
        sgi = ((st // NS) * NSP + st % NS)[order].astype(np.int32)
        sw = wn[order].astype(BF16)
        # group boundaries (sorted by core, rel, loc)
        bnd = np.nonzero(np.diff(skey))[0] + 1
        gstart = np.concatenate([[0], bnd]).astype(np.int64)
        gend = np.concatenate([bnd, [skey.size]]).astype(np.int64)
        gcnt = gend - gstart
        gkey = skey[gstart]
        gc = gkey // (R * NSP)
        grem = gkey - gc * (R * NSP)
        gr = grem // NSP
        gloc = grem - gr * NSP
        gdst = gloc * 4 + gr
        # split groups into windows: one main (<=GBIG) + overflow chunks
        nw = 1 + (np.maximum(gcnt - GBIG, 0) + GBIG - 1) // GBIG
        wg = np.repeat(np.arange(gcnt.size), nw)
        wi = _segment_arange(nw)
        wstart = gstart[wg] + wi * GBIG
        wsize = np.minimum(gcnt[wg] - wi * GBIG, GBIG)
        wover = wi > 0
        wc = gc[wg]
        wdst = gdst[wg]
        wcls = np.searchsorted(np.asarray(GCL), wsize)

        gis = [[] for _ in range(NC)]
        scs = [[] for _ in range(NC)]
        sds = [[] for _ in range(NC)]

        def emit(sel_per_core, G, is_over):
            ngmax = max(s.size for s in sel_per_core)
            if ngmax == 0:
                return
            gpt = (128 * KTILE) // G
            ntiles = -(-ngmax // gpt)
            for ti in range(ntiles):
                glo, ghi = ti * gpt, min(ngmax, (ti + 1) * gpt)
                ng_pad = ghi - glo
                k = -(-(ng_pad * G) // 128)
                k = -(-k // G) * G
                q = k // G
                ngt = 128 * q
                tiles[t].append(dict(G=G, k=k, q=q, over=is_over))
                for cc in range(NC):
                    sel = sel_per_core[cc][glo:ghi]
                    giA = np.zeros(128 * k, np.int32)
                    scA = np.zeros(128 * k, BF16)
                    sdA = np.full(ngt, DUMMY, np.int32)
                    sdA[: sel.size] = wdst[sel]
                    sizes = wsize[sel]
                    within = _segment_arange(sizes)
                    slots = np.repeat(np.arange(sel.size) * G, sizes) + within
                    epos = np.repeat(wstart[sel], sizes) + within
                    giA[slots] = sgi[epos]
                    scA[slots] = sw[epos]
                    gis[cc].append(giA.reshape(128, k))
                    scs[cc].append(scA.reshape(128, k))
                    sds[cc].append(sdA.reshape(128, q))

        for ci, G in enumerate(GCL):
            m = (~wover) & (wcls == ci)
            emit([np.nonzero(m & (wc == cc))[0] for cc in range(NC)], G, False)
        emit([np.nonzero(wover & (wc == cc))[0] for cc in range(NC)],
             GBIG, True)

        for cc in range(NC):
            per_core[cc][f"gi_{t}"] = (np.concatenate(gis[cc], 1)
                                       if gis[cc] else np.zeros((128, 0), np.int32))
            per_core[cc][f"sc_{t}"] = (np.concatenate(scs[cc], 1)
                                       if scs[cc] else np.zeros((128, 0), BF16))
            per_core[cc][f"sd_{t}"] = (np.concatenate(sds[cc], 1)
                                       if sds[cc] else np.zeros((128, 0), np.int32))
    return per_core, tiles


def make_feat_shards(feat):
    feat = np.asarray(feat, np.float32).reshape(T, NC, NS, D)
    x = np.zeros((NC, T, NSP, D), BF16)
    for cc in range(NC):
        x[cc, :, :NS, :] = feat[:, cc].astype(BF16)
    return [np.ascontiguousarray(x[cc]) for cc in range(NC)]


def make_weights(W1, b1, W2, b2, Wih, Whh, bih, bhh, Wc1, bc1, Wc2, bc2):
    f = np.float32

    def bf(x):
        return np.ascontiguousarray(np.asarray(x, f)).astype(BF16)

    W1 = np.asarray(W1, f)
    W2 = np.asarray(W2, f)
    b1s = np.asarray(b1, f).sum(0)[None, :]
    b2s = np.asarray(b2, f).sum(0)[None, :]
    z63 = np.zeros((63, D), f)
    Wih = np.asarray(Wih, f)
    Whh = np.asarray(Whh, f)
    bih = np.asarray(bih, f)
    bhh = np.asarray(bhh, f)
    return dict(
        w1s=bf(np.vstack([W1[0], W1[1]])),
        w1r2=bf(np.vstack([W1[2], b1s, z63])),
        w2s=bf(np.vstack([W2[0], W2[1]])),
        w2r2=bf(np.vstack([W2[2], b2s, z63])),
        wihrz=Wih[0:2 * D].T.copy(),
        whhrz=Whh[0:2 * D].T.copy(),
        wihn=Wih[2 * D:3 * D].T.copy(),
        whhn=Whh[2 * D:3 * D].T.copy(),
        gbr=(bih + bhh)[0:D][:, None].copy(),
        gbz=(bih + bhh)[D:2 * D][:, None].copy(),
        gbhn=bhh[2 * D:][:, None].copy(),
        gbin=bih[2 * D:][:, None].copy(),
        wc1=np.asarray(Wc1, f).copy(),
        bc1c=np.asarray(bc1, f)[:, None].copy(),
        wc2=np.asarray(Wc2, f).copy(),
        bc2c=np.asarray(bc2, f).reshape(1, 1).copy(),
    )


W_SHAPES = dict(
    w1s=([2 * D, D], "bf16"), w1r2=([2 * D, D], "bf16"),
    w2s=([2 * D, D], "bf16"), w2r2=([2 * D, D], "bf16"),
    wihrz=([D, 2 * D], "f32"), whhrz=([D, 2 * D], "f32"),
    wihn=([D, D], "f32"), whhn=([D, D], "f32"),
    gbr=([D, 1], "f32"), gbz=([D, 1], "f32"),
    gbhn=([D, 1], "f32"), gbin=([D, 1], "f32"),
    wc1=([D, D], "f32"), bc1c=([D, 1], "f32"),
    wc2=([D, 1], "f32"), bc2c=([1, 1], "f32"),
)


# --------------------------------------------------------------------------
# device program
# --------------------------------------------------------------------------

def build_program(tiles):
    from concourse import bacc, bass, mybir, tile

    f32 = mybir.dt.float32
    i32 = mybir.dt.int32
    bf16 = mybir.dt.bfloat16
    ALU = mybir.AluOpType
    AF = mybir.ActivationFunctionType
    DT = {"f32": f32, "bf16": bf16}

    nc = bacc.Bacc("TRN2", target_bir_lowering=False, debug=False)

    xsh_d = nc.dram_tensor("xsh", [T, NSP, D], bf16, kind="ExternalInput")
    gi_d, sc_d, sd_d = {}, {}, {}
    for t in range(T):
        ck = sum(td["k"] for td in tiles[t])
        cq = sum(td["q"] for td in tiles[t])
        gi_d[t] = nc.dram_tensor(f"gi_{t}", [128, ck], i32, kind="ExternalInput")
        sc_d[t] = nc.dram_tensor(f"sc_{t}", [128, ck], bf16, kind="ExternalInput")
        sd_d[t] = nc.dram_tensor(f"sd_{t}", [128, cq], i32, kind="ExternalInput")
    w_d = {nm: nc.dram_tensor(nm, sh, DT[dt], kind="ExternalInput")
           for nm, (sh, dt) in W_SHAPES.items()}
    out_d = nc.dram_tensor("out", [1, NS], f32, kind="ExternalOutput")

    featf = [nc.dram_tensor(f"featf{t}", [NV2, D], bf16, addr_space="Shared")
             for t in range(T)]
    xin = [nc.dram_tensor(f"xin{t}", [NSP, D], bf16) for t in range(T)]
    agg_d = [nc.dram_tensor(f"agg{i}", [AGG_ROWS, D], bf16) for i in range(2)]
    ag_in = [nc.dram_tensor(f"agin{t}", [NSP, D], bf16) for t in range(T)]
    h1f = [nc.dram_tensor(f"h1f{t}", [NV2, D], bf16, addr_space="Shared")
           for t in range(T)]
    h2T = [nc.dram_tensor(f"h2T{t}", [D, NSP], f32) for t in range(T)]

    with tile.TileContext(nc) as tc:
        with tc.tile_pool(name="const", bufs=1) as cpool:
            wt = {nm: cpool.tile(sh, DT[dt], tag=nm, name="w_" + nm)
                  for nm, (sh, dt) in W_SHAPES.items()}
            for nm in W_SHAPES:
                nc.sync.dma_start(wt[nm][:], w_d[nm][:])
            # zero tile with the z-lane 1.0 marker at feature offset 192
            zt = cpool.tile([128, 25, 256], bf16, tag="zt")
            nc.vector.memset(zt[:], 0.0)
            nc.vector.memset(zt[:, :, 192:193], 1.0)

            def edge_phase(t, src_d, agg, pool):
                for zci in range(4):
                    lo = zci * 3200
                    nc.sync.dma_start(
                        agg[lo * 4:(lo + 3200) * 4].rearrange(
                            "(j p h) d -> p j (h d)", p=128, h=4),
                        zt[:])
                kofs = qofs = 0
                for td in tiles[t]:
                    G, k, q = td["G"], td["k"], td["q"]
                    git = pool.tile([128, k], i32, tag="git")
                    nc.scalar.dma_start(git[:], gi_d[t][:, kofs:kofs + k])
                    sct = pool.tile([128, k], bf16, tag="sct")
                    nc.scalar.dma_start(sct[:], sc_d[t][:, kofs:kofs + k])
                    sdt = pool.tile([128, q], i32, tag="sdt")
                    nc.scalar.dma_start(sdt[:], sd_d[t][:, qofs:qofs + q])
                    msgs = pool.tile([128, k, D], bf16, tag="msgs")
                    for j in range(k):
                        nc.gpsimd.indirect_dma_start(
                            out=msgs[:, j, :], out_offset=None, in_=src_d[:],
                            in_offset=bass.IndirectOffsetOnAxis(
                                ap=git[:, j:j + 1], axis=0))
                    nc.vector.tensor_tensor(
                        out=msgs[:], in0=msgs[:],
                        in1=sct[:, :, None].to_broadcast([128, k, D]),
                        op=ALU.mult)
                    grp = pool.tile([128, q, D], bf16, tag="grp")
                    with nc.allow_low_precision(reason="bf16 group rows"):
                        nc.vector.tensor_reduce(
                            out=grp[:],
                            in_=msgs[:].rearrange("p (q g) d -> p q d g", g=G),
                            axis=mybir.AxisListType.X, op=ALU.add)
                    op = ALU.add if td["over"] else ALU.bypass
                    for jq in range(q):
                        nc.gpsimd.indirect_dma_start(
                            out=agg[:],
                            out_offset=bass.IndirectOffsetOnAxis(
                                ap=sdt[:, jq:jq + 1], axis=0),
                            in_=grp[:, jq, :], in_offset=None,
                            compute_op=op)
                    kofs += k
                    qofs += q

            def load_aT(agg, c0, pool):
                aggv = agg[:].rearrange("(n h) d -> n (h d)", h=4)
                at01 = pool.tile([128, MCH], bf16, tag="at01")
                at2z = pool.tile([128, MCH], bf16, tag="at2z")
                nc.sync.dma_start(at01[:], aggv[c0:c0 + MCH, 0:128],
                                  transpose=True)
                nc.sync.dma_start(at2z[:], aggv[c0:c0 + MCH, 128:256],
                                  transpose=True)
                return at01, at2z

            def dense1(t, agg, pool, psum):
                for m in range(NCHK):
                    c0 = m * MCH
                    at01, at2z = load_aT(agg, c0, pool)
                    po = psum.tile([128, 4, D], f32, tag="po1", space="PSUM")
                    for j in range(4):
                        nc.tensor.matmul(po[:, j, :],
                                         at01[:, j * 128:(j + 1) * 128],
                                         wt["w1s"][:], start=True, stop=False)
                        nc.tensor.matmul(po[:, j, :],
                                         at2z[:, j * 128:(j + 1) * 128],
                                         wt["w1r2"][:], start=False, stop=True)
                    h1t = pool.tile([128, 4, D], bf16, tag="h1t")
                    nc.scalar.activation(h1t[:], po[:], AF.Relu)
                    nc.sync.dma_start(
                        ag_in[t][c0:c0 + MCH].rearrange("(j p) d -> p j d",
                                                        p=128),
                        h1t[:])

            def dense2(t, agg, pool, psum):
                for m in range(NCHK):
                    c0 = m * MCH
                    at01, at2z = load_aT(agg, c0, pool)
                    po2 = psum.tile([D, MCH], f32, tag="po2", space="PSUM")
                    nc.tensor.matmul(po2[:], wt["w2s"][:], at01[:],
                                     start=True, stop=False)
                    nc.tensor.matmul(po2[:], wt["w2r2"][:], at2z[:],
                                     start=False, stop=True)
                    h2t = pool.tile([D, MCH], f32, tag="h2t")
                    nc.scalar.activation(h2t[:], po2[:], AF.Relu)
                    nc.sync.dma_start(h2T[t][:, c0:c0 + MCH], h2t[:])

            with (
                tc.tile_pool(name="work", bufs=2) as pool,
                tc.tile_pool(name="ps", bufs=2, space="PSUM") as psum,
            ):
                for t in range(T):
                    # collectives cannot read IO tensors: stage via SBUF
                    xst = pool.tile([128, NSP // 128, D], bf16, tag="xst")
                    nc.sync.dma_start(
                        xst[:], xsh_d[t].rearrange("(j p) d -> p j d", p=128))
                    nc.sync.dma_start(
                        xin[t][:].rearrange("(j p) d -> p j d", p=128), xst[:])
                for t in range(T):
                    nc.gpsimd.collective_compute(
                        "AllGather", ALU.bypass,
                        replica_groups=[list(range(NC))],
                        ins=[xin[t][:]], outs=[featf[t][:]])
                for t in range(T):
                    agg = agg_d[t % 2]
                    edge_phase(t, featf[t], agg, pool)
                    dense1(t, agg, pool, psum)
                    nc.gpsimd.collective_compute(
                        "AllGather", ALU.bypass,
                        replica_groups=[list(range(NC))],
                        ins=[ag_in[t][:]], outs=[h1f[t][:]])
                for t in range(T):
                    agg = agg_d[t % 2]
                    edge_phase(t, h1f[t], agg, pool)
                    dense2(t, agg, pool, psum)

            # ---- GRU + MLP over feature-major chunks
            with (
                tc.tile_pool(name="gw", bufs=2) as gpool,
                tc.tile_pool(name="gp", bufs=1, space="PSUM") as gps,
            ):
                lrow = gpool.tile([1, NSP], f32, tag="lrow")
                for m in range(NCHK):
                    cols = slice(m * MCH, (m + 1) * MCH)
                    hA = gpool.tile([D, MCH], f32, tag="hA")
                    hB = gpool.tile([D, MCH], f32, tag="hB")
                    nc.vector.memset(hA[:], 0.0)
                    for t in range(T):
                        hin = hA if t % 2 == 0 else hB
                        hout = hB if t % 2 == 0 else hA
                        xT = gpool.tile([D, MCH], f32, tag="xT")
                        nc.sync.dma_start(xT[:], h2T[t][:, cols])
                        ps_r = gps.tile([D, MCH], f32, tag="ps_r", space="PSUM")
                        nc.tensor.matmul(ps_r[:], wt["wihrz"][:, 0:D], xT[:],
                                         start=True, stop=False)
                        nc.tensor.matmul(ps_r[:], wt["whhrz"][:, 0:D], hin[:],
                                         start=False, stop=True)
                        ps_z = gps.tile([D, MCH], f32, tag="ps_z", space="PSUM")
                        nc.tensor.matmul(ps_z[:], wt["wihrz"][:, D:2 * D],
                                         xT[:], start=True, stop=False)
                        nc.tensor.matmul(ps_z[:], wt["whhrz"][:, D:2 * D],
                                         hin[:], start=False, stop=True)
                        ps_n = gps.tile([D, MCH], f32, tag="ps_n", space="PSUM")
                        nc.tensor.matmul(ps_n[:], wt["wihn"][:], xT[:],
                                         start=True, stop=True)
                        ps_h = gps.tile([D, MCH], f32, tag="ps_h", space="PSUM")
                        nc.tensor.matmul(ps_h[:], wt["whhn"][:], hin[:],
                                         start=True, stop=True)
                        r_sb = gpool.tile([D, MCH], f32, tag="r_sb")
                        nc.scalar.activation(r_sb[:], ps_r[:], AF.Sigmoid,
                                             bias=wt["gbr"][:])
                        z_sb = gpool.tile([D, MCH], f32, tag="z_sb")
                        nc.scalar.activation(z_sb[:], ps_z[:], AF.Sigmoid,
                                             bias=wt["gbz"][:])
                        hn = gpool.tile([D, MCH], f32, tag="hn")
                        nc.scalar.activation(hn[:], ps_h[:], AF.Identity,
                                             bias=wt["gbhn"][:])
                        nc.vector.tensor_tensor(out=hn[:], in0=r_sb[:],
                                                in1=hn[:], op=ALU.mult)
                        nc.vector.tensor_tensor(out=hn[:], in0=ps_n[:],
                                                in1=hn[:], op=ALU.add)
                        nt = gpool.tile([D, MCH], f32, tag="nt")
                        nc.scalar.activation(nt[:], hn[:], AF.Tanh,
                                             bias=wt["gbin"][:])
                        nc.vector.tensor_tensor(out=hout[:], in0=hin[:],
                                                in1=nt[:], op=ALU.subtract)
                        nc.vector.tensor_tensor(out=hout[:], in0=z_sb[:],
                                                in1=hout[:], op=ALU.mult)
                        nc.vector.tensor_tensor(out=hout[:], in0=nt[:],
                                                in1=hout[:], op=ALU.add)
                    hlast = hA if T % 2 == 0 else hB
                    ps_f = gps.tile([D, MCH], f32, tag="ps_f", space="PSUM")
                    nc.tensor.matmul(ps_f[:], wt["wc1"][:], hlast[:],
                                     start=True, stop=True)
                    zf = gpool.tile([D, MCH], f32, tag="zf")
                    nc.scalar.activation(zf[:], ps_f[:], AF.Relu,
                                         bias=wt["bc1c"][:])
                    ps_l = gps.tile([1, MCH], f32, tag="ps_l", space="PSUM")
                    nc.tensor.matmul(ps_l[:], wt["wc2"][:], zf[:],
                                     start=True, stop=True)
                    nc.scalar.activation(lrow[:, cols], ps_l[:], AF.Identity,
                                         bias=wt["bc2c"][:])
                nc.sync.dma_start(out_d[:], lrow[:, 0:NS])

    nc.compile()
    return nc


# --------------------------------------------------------------------------
# fast PJRT runner: pre-place sharded inputs with device_put so the jitted
# shard_map call doesn't re-transfer unsharded numpy args through the relay
# --------------------------------------------------------------------------

def _install_fast_pjrt_runner():
    import jax
    from jax.sharding import Mesh, PartitionSpec, NamedSharding
    from jax.experimental.shard_map import shard_map
    from concourse import bass2jax, mybir
    from concourse.bass2jax import (_bass_exec_p, install_neuronx_cc_hook,
                                    partition_id_tensor)

    def run_fast(nc, in_maps, n_cores):
        install_neuronx_cc_hook()
        partition_name = (nc.partition_id_tensor.name
                          if nc.partition_id_tensor else None)
        in_names, out_names, out_avals, zero_outs = [], [], [], []
        for alloc in nc.m.functions[0].allocations:
            if not isinstance(alloc, mybir.MemoryLocationSet):
                continue
            name = alloc.memorylocations[0].name
            if alloc.kind == "ExternalInput":
                if name != partition_name:
                    in_names.append(name)
            elif alloc.kind == "ExternalOutput":
                out_names.append(name)
                shape = tuple(alloc.tensor_shape)
                dtype = mybir.dt.np(alloc.dtype)
                out_avals.append(jax.core.ShapedArray(shape, dtype))
                zero_outs.append(np.zeros(shape, dtype))
        n_params = len(in_names)
        in_names.extend(out_names)
        if partition_name is not None:
            in_names.append(partition_name)

        def _body(*args):
            operands = list(args)
            if partition_name is not None:
                operands.append(partition_id_tensor())
            outs = _bass_exec_p.bind(
                *operands, out_avals=tuple(out_avals),
                in_names=tuple(in_names), out_names=tuple(out_names),
                lowering_input_output_aliases=(),
                sim_require_finite=True, sim_require_nnan=True, nc=nc)
            return tuple(outs)

        devices = jax.devices()[:n_cores]
        mesh = Mesh(np.asarray(devices), ("core",))
        n_outs = len(out_avals)
        in_specs = (PartitionSpec("core"),) * (n_params + n_outs)
        out_specs = (PartitionSpec("core"),) * len(out_names)
        sharded = jax.jit(
            shard_map(_body, mesh=mesh, in_specs=in_specs,
                      out_specs=out_specs, check_rep=False),
            keep_unused=True)
        concat_in = [
            np.concatenate([np.asarray(m[name]) for m in in_maps], axis=0)
            for name in in_names[:n_params]]
        concat_zeros = [
            np.zeros((n_cores * z.shape[0], *z.shape[1:]), z.dtype)
            for z in zero_outs]
        import time as _time
        _t0 = _time.monotonic()
        sh = NamedSharding(mesh, PartitionSpec("core"))
        dev_in = [jax.device_put(a, sh) for a in concat_in]
        dev_zero = [jax.device_put(z, sh) for z in concat_zeros]
        for a in dev_in:
            a.block_until_ready()
        _t1 = _time.monotonic()
        out_arrs = sharded(*dev_in, *dev_zero)
        for o in out_arrs:
            o.block_until_ready()
        _t2 = _time.monotonic()
        print(f"[kernel] upload {_t1-_t0:.1f}s  exec {_t2-_t1:.1f}s",
              file=sys.stderr, flush=True)
        return [
            {name: np.asarray(out_arrs[i]).reshape(
                n_cores, *out_avals[i].shape)[c]
             for i, name in enumerate(out_names)}
            for c in range(n_cores)
        ]

    bass2jax.run_bass_via_pjrt = run_fast


# --------------------------------------------------------------------------
# entry point
# --------------------------------------------------------------------------

def kernel(**inputs):
    import time

    per_core, tiles = preprocess(inputs["src"], inputs["dst"], inputs["ew"])
    wts = make_weights(
        inputs["W1"], inputs["b1"], inputs["W2"], inputs["b2"],
        inputs["Wih"], inputs["Whh"], inputs["bih"], inputs["bhh"],
        inputs["Wc1"], inputs["bc1"], inputs["Wc2"], inputs["bc2"])
    shards = make_feat_shards(inputs["feat"])
    nc = build_program(tiles)

    in_maps = []
    for cc in range(NC):
        m = dict(per_core[cc])
        m.update(wts)
        m["xsh"] = shards[cc]
        in_maps.append(m)

    _install_fast_pjrt_runner()
    from concourse.bass_utils import run_bass_kernel_spmd
    kwargs = {}
    if TRACE:
        kwargs = dict(trace=True, trace_cores=list(range(NC)))
    t0 = time.monotonic()
    try:
        res = run_bass_kernel_spmd(nc, in_maps, list(range(NC)), **kwargs)
    except (ImportError, ModuleNotFoundError):
        # NTFF profiling hook unavailable in this environment
        res = run_bass_kernel_spmd(nc, in_maps, list(range(NC)))
    wall_ns = (time.monotonic() - t0) * 1e9
    global LAST_EXEC_NS
    LAST_EXEC_NS = res.exec_time_ns if res.exec_time_ns else int(wall_ns)
    out = np.concatenate(
        [np.asarray(res.results[cc]["out"]).reshape(NS) for cc in range(NC)])
    return out.astype(np.float32)


if __name__ == "__main__":
    pass


# revision 10
# speedup vs baseline: 14.5826x; 1.1127x over previous
"""Trainium2 Bass kernel for the DGL-style heterogeneous temporal GNN (v2).

Model (per reference):
  for t: h1 = relu(sum_r GraphConv_r(feat[t])); h2 = relu(sum_r GraphConv_r(h1))
  h_last = GRU over t of h2; logits = MLP(h_last)

v2 design (vs. v1 baseline):
 - ship ~25x fewer bytes: feat shards bf16 (device AllGather replicates),
   per-slot gather tables gi (i32 padded src row) + sc (bf16 w/deg).
 - inputs pre-placed with device_put(NamedSharding) so the jitted shard_map
   call doesn't re-transfer unsharded numpy args through the axon relay.
 - bf16 message/aggregate datapath (fp32 accumulation inside DVE reduce and
   PSUM): enables DMA-transpose ([rows,128]bf16 -> [128,rows]) so the dense
   conv phases need no TensorE transposes.
 - agg layout [NSP, 4, 64]: relations r0,r1,r2 + a "z" lane whose feature 0
   is pre-set to 1.0, so conv bias rides in the weight matrix (row 64).
"""
import sys

sys.path.insert(0, "/opt/trn_rl_repo")
import numpy as np
import ml_dtypes

BF16 = ml_dtypes.bfloat16
TRACE = False
LAST_EXEC_NS = None

T, R, N, E, D = 4, 3, 100000, 800000, 64
NC = 8
NS = N // NC            # 12500
NSP = 12800             # padded shard rows
NV2 = NC * NSP          # 102400
KTILE = 256             # max slot columns per gather tile
GCL = (2, 4, 6, 8, 10, 12, 14, 16, 20, 24, 32, 64)
GBIG = 64
DUMMY = NS * 4          # scatter row for padded groups (pad region)
AGG_ROWS = NSP * 4
MCH = 512               # dense-phase chunk (locs)
NCHK = NSP // MCH       # 25


def _segment_arange(sizes):
    total = int(sizes.sum())
    if total == 0:
        return np.zeros(0, np.int64)
    starts = np.cumsum(sizes) - sizes
    return np.arange(total) - np.repeat(starts, sizes)


# --------------------------------------------------------------------------
# host-side preprocessing
# --------------------------------------------------------------------------

def preprocess(src, dst, ew):
    """Per-core edge tables + shared tile descriptors.

    Returns (per_core, tiles): per_core[c] = {gi_t: [128,CK] i32,
    sc_t: [128,CK] bf16, sd_t: [128,CQ] i32}; tiles[t] = list of
    dict(G,k,q,over) shared by all cores (SPMD-uniform).
    """
    src = np.asarray(src, np.int64).reshape(T, R * E)
    dst = np.asarray(dst, np.int64).reshape(T, R * E)
    ew = np.asarray(ew, np.float32).reshape(T, R * E)
    r_of_e = np.repeat(np.arange(R, dtype=np.int64), E)
    tiles = {t: [] for t in range(T)}
    per_core = [dict() for _ in range(NC)]

    for t in range(T):
        st, dt_, w_t = src[t], dst[t], ew[t]
        deg = np.bincount(r_of_e * N + dt_, minlength=R * N)
        wn = w_t / np.maximum(deg[r_of_e * N + dt_], 1).astype(np.float32)
        c = dt_ // NS
        loc = dt_ - c * NS
        key = (c * R + r_of_e) * NSP + loc
        order = np.argsort(key, kind="stable")
        skey = key[order]
        sgi = ((st // NS) * NSP + st % NS)[order].astype(np.int32)
        sw = wn[order].astype(BF16)
        # group boundaries (sorted by core, rel, loc)
        bnd = np.nonzero(np.diff(skey))[0] + 1
        gstart = np.concatenate([[0], bnd]).astype(np.int64)
        gend = np.concatenate([bnd, [skey.size]]).astype(np.int64)
        gcnt = gend - gstart
        gkey = skey[gstart]
        gc = gkey // (R * NSP)
        grem = gkey - gc * (R * NSP)
        gr = grem // NSP
        gloc = grem - gr * NSP
        gdst = gloc * 4 + gr
        # split groups into windows: one main (<=GBIG) + overflow chunks
        nw = 1 + (np.maximum(gcnt - GBIG, 0) + GBIG - 1) // GBIG
        wg = np.repeat(np.arange(gcnt.size), nw)
        wi = _segment_arange(nw)
        wstart = gstart[wg] + wi * GBIG
        wsize = np.minimum(gcnt[wg] - wi * GBIG, GBIG)
        wover = wi > 0
        wc = gc[wg]
        wdst = gdst[wg]
        wcls = np.searchsorted(np.asarray(GCL), wsize)

        gis = [[] for _ in range(NC)]
        scs = [[] for _ in range(NC)]
        sds = [[] for _ in range(NC)]

        def emit(sel_per_core, G, is_over):
            ngmax = max(s.size for s in sel_per_core)
            if ngmax == 0:
                return
            gpt = (128 * KTILE) // G
            ntiles = -(-ngmax // gpt)
            for ti in range(ntiles):
                glo, ghi = ti * gpt, min(ngmax, (ti + 1) * gpt)
                ng_pad = ghi - glo
                k = -(-(ng_pad * G) // 128)
                k = -(-k // G) * G
                q = k // G
                ngt = 128 * q
                tiles[t].append(dict(G=G, k=k, q=q, over=is_over))
                for cc in range(NC):
                    sel = sel_per_core[cc][glo:ghi]
                    giA = np.zeros(128 * k, np.int32)
                    scA = np.zeros(128 * k, BF16)
                    sdA = np.full(ngt, DUMMY, np.int32)
                    sdA[: sel.size] = wdst[sel]
                    sizes = wsize[sel]
                    within = _segment_arange(sizes)
                    slots = np.repeat(np.arange(sel.size) * G, sizes) + within
                    epos = np.repeat(wstart[sel], sizes) + within
                    giA[slots] = sgi[epos]
                    scA[slots] = sw[epos]
                    gis[cc].append(giA.reshape(128, k))
                    scs[cc].append(scA.reshape(128, k))
                    sds[cc].append(sdA.reshape(128, q))

        for ci, G in enumerate(GCL):
            m = (~wover) & (wcls == ci)
            emit([np.nonzero(m & (wc == cc))[0] for cc in range(NC)], G, False)
        emit([np.nonzero(wover & (wc == cc))[0] for cc in range(NC)],
             GBIG, True)

        for cc in range(NC):
            per_core[cc][f"gi_{t}"] = (np.concatenate(gis[cc], 1)
                                       if gis[cc] else np.zeros((128, 0), np.int32))
            per_core[cc][f"sc_{t}"] = (np.concatenate(scs[cc], 1)
                                       if scs[cc] else np.zeros((128, 0), BF16))
            per_core[cc][f"sd_{t}"] = (np.concatenate(sds[cc], 1)
                                       if sds[cc] else np.zeros((128, 0), np.int32))
    return per_core, tiles


def make_feat_shards(feat):
    feat = np.asarray(feat, np.float32).reshape(T, NC, NS, D)
    x = np.zeros((NC, T, NSP, D), BF16)
    for cc in range(NC):
        x[cc, :, :NS, :] = feat[:, cc].astype(BF16)
    return [np.ascontiguousarray(x[cc]) for cc in range(NC)]


def make_weights(W1, b1, W2, b2, Wih, Whh, bih, bhh, Wc1, bc1, Wc2, bc2):
    f = np.float32

    def bf(x):
        return np.ascontiguousarray(np.asarray(x, f)).astype(BF16)

    W1 = np.asarray(W1, f)
    W2 = np.asarray(W2, f)
    b1s = np.asarray(b1, f).sum(0)[None, :]
    b2s = np.asarray(b2, f).sum(0)[None, :]
    z63 = np.zeros((63, D), f)
    Wih = np.asarray(Wih, f)
    Whh = np.asarray(Whh, f)
    bih = np.asarray(bih, f)
    bhh = np.asarray(bhh, f)
    return dict(
        w1s=bf(np.vstack([W1[0], W1[1]])),
        w1r2=bf(np.vstack([W1[2], b1s, z63])),
        w2s=bf(np.vstack([W2[0], W2[1]])),
        w2r2=bf(np.vstack([W2[2], b2s, z63])),
        wihrz=Wih[0:2 * D].T.copy(),
        whhrz=Whh[0:2 * D].T.copy(),
        wihn=Wih[2 * D:3 * D].T.copy(),
        whhn=Whh[2 * D:3 * D].T.copy(),
        gbr=(bih + bhh)[0:D][:, None].copy(),
        gbz=(bih + bhh)[D:2 * D][:, None].copy(),
        gbhn=bhh[2 * D:][:, None].copy(),
        gbin=bih[2 * D:][:, None].copy(),
        wc1=np.asarray(Wc1, f).copy(),
        bc1c=np.asarray(bc1, f)[:, None].copy(),
        wc2=np.asarray(Wc2, f).copy(),
        bc2c=np.asarray(bc2, f).reshape(1, 1).copy(),
    )


W_SHAPES = dict(
    w1s=([2 * D, D], "bf16"), w1r2=([2 * D, D], "bf16"),
    w2s=([2 * D, D], "bf16"), w2r2=([2 * D, D], "bf16"),
    wihrz=([D, 2 * D], "f32"), whhrz=([D, 2 * D], "f32"),
    wihn=([D, D], "f32"), whhn=([D, D], "f32"),
    gbr=([D, 1], "f32"), gbz=([D, 1], "f32"),
    gbhn=([D, 1], "f32"), gbin=([D, 1], "f32"),
    wc1=([D, D], "f32"), bc1c=([D, 1], "f32"),
    wc2=([D, 1], "f32"), bc2c=([1, 1], "f32"),
)


# --------------------------------------------------------------------------
# device program
# --------------------------------------------------------------------------

def build_program(tiles):
    from concourse import bacc, bass, mybir, tile

    f32 = mybir.dt.float32
    i32 = mybir.dt.int32
    bf16 = mybir.dt.bfloat16
    ALU = mybir.AluOpType
    AF = mybir.ActivationFunctionType
    DT = {"f32": f32, "bf16": bf16}

    nc = bacc.Bacc("TRN2", target_bir_lowering=False, debug=False)

    xsh_d = nc.dram_tensor("xsh", [T, NSP, D], bf16, kind="ExternalInput")
    gi_d, sc_d, sd_d = {}, {}, {}
    for t in range(T):
        ck = sum(td["k"] for td in tiles[t])
        cq = sum(td["q"] for td in tiles[t])
        gi_d[t] = nc.dram_tensor(f"gi_{t}", [128, ck], i32, kind="ExternalInput")
        sc_d[t] = nc.dram_tensor(f"sc_{t}", [128, ck], bf16, kind="ExternalInput")
        sd_d[t] = nc.dram_tensor(f"sd_{t}", [128, cq], i32, kind="ExternalInput")
    w_d = {nm: nc.dram_tensor(nm, sh, DT[dt], kind="ExternalInput")
           for nm, (sh, dt) in W_SHAPES.items()}
    out_d = nc.dram_tensor("out", [1, NS], f32, kind="ExternalOutput")

    featf = [nc.dram_tensor(f"featf{t}", [NV2, D], bf16, addr_space="Shared")
             for t in range(T)]
    xin = [nc.dram_tensor(f"xin{t}", [NSP, D], bf16) for t in range(T)]
    agg_d = [nc.dram_tensor(f"agg{i}", [AGG_ROWS, D], bf16) for i in range(2)]
    ag_in = [nc.dram_tensor(f"agin{t}", [NSP, D], bf16) for t in range(T)]
    h1f = [nc.dram_tensor(f"h1f{t}", [NV2, D], bf16, addr_space="Shared")
           for t in range(T)]
    h2T = [nc.dram_tensor(f"h2T{t}", [D, NSP], f32) for t in range(T)]

    with tile.TileContext(nc) as tc:
        with tc.tile_pool(name="const", bufs=1) as cpool:
            wt = {nm: cpool.tile(sh, DT[dt], tag=nm, name="w_" + nm)
                  for nm, (sh, dt) in W_SHAPES.items()}
            for nm in W_SHAPES:
                nc.sync.dma_start(wt[nm][:], w_d[nm][:])
            # zero tile with the z-lane 1.0 marker at feature offset 192
            zt = cpool.tile([128, 25, 256], bf16, tag="zt")
            nc.vector.memset(zt[:], 0.0)
            nc.vector.memset(zt[:, :, 192:193], 1.0)

            def edge_phase(t, src_d, agg, pool):
                for zci in range(4):
                    lo = zci * 3200
                    nc.sync.dma_start(
                        agg[lo * 4:(lo + 3200) * 4].rearrange(
                            "(j p h) d -> p j (h d)", p=128, h=4),
                        zt[:])
                kofs = qofs = 0
                for td in tiles[t]:
                    G, k, q = td["G"], td["k"], td["q"]
                    git = pool.tile([128, k], i32, tag="git")
                    nc.scalar.dma_start(git[:], gi_d[t][:, kofs:kofs + k])
                    sct = pool.tile([128, k], bf16, tag="sct")
                    nc.scalar.dma_start(sct[:], sc_d[t][:, kofs:kofs + k])
                    sdt = pool.tile([128, q], i32, tag="sdt")
                    nc.scalar.dma_start(sdt[:], sd_d[t][:, qofs:qofs + q])
                    msgs = pool.tile([128, k, D], bf16, tag="msgs")
                    for j in range(k):
                        nc.gpsimd.indirect_dma_start(
                            out=msgs[:, j, :], out_offset=None, in_=src_d[:],
                            in_offset=bass.IndirectOffsetOnAxis(
                                ap=git[:, j:j + 1], axis=0))
                    nc.vector.tensor_tensor(
                        out=msgs[:], in0=msgs[:],
                        in1=sct[:, :, None].to_broadcast([128, k, D]),
                        op=ALU.mult)
                    grp = pool.tile([128, q, D], bf16, tag="grp")
                    with nc.allow_low_precision(reason="bf16 group rows"):
                        nc.vector.tensor_reduce(
                            out=grp[:],
                            in_=msgs[:].rearrange("p (q g) d -> p q d g", g=G),
                            axis=mybir.AxisListType.X, op=ALU.add)
                    op = ALU.add if td["over"] else ALU.bypass
                    for jq in range(q):
                        nc.gpsimd.indirect_dma_start(
                            out=agg[:],
                            out_offset=bass.IndirectOffsetOnAxis(
                                ap=sdt[:, jq:jq + 1], axis=0),
                            in_=grp[:, jq, :], in_offset=None,
                            compute_op=op)
                    kofs += k
                    qofs += q

            def load_aT(agg, c0, pool):
                aggv = agg[:].rearrange("(n h) d -> n (h d)", h=4)
                at01 = pool.tile([128, MCH], bf16, tag="at01")
                at2z = pool.tile([128, MCH], bf16, tag="at2z")
                nc.sync.dma_start(at01[:], aggv[c0:c0 + MCH, 0:128],
                                  transpose=True)
                nc.sync.dma_start(at2z[:], aggv[c0:c0 + MCH, 128:256],
                                  transpose=True)
                return at01, at2z

            def dense1(t, agg, pool, psum):
                for m in range(NCHK):
                    c0 = m * MCH
                    at01, at2z = load_aT(agg, c0, pool)
                    po = psum.tile([128, 4, D], f32, tag="po1", space="PSUM")
                    for j in range(4):
                        nc.tensor.matmul(po[:, j, :],
                                         at01[:, j * 128:(j + 1) * 128],
                                         wt["w1s"][:], start=True, stop=False)
                        nc.tensor.matmul(po[:, j, :],
                                         at2z[:, j * 128:(j + 1) * 128],
                                         wt["w1r2"][:], start=False, stop=True)
                    h1t = pool.tile([128, 4, D], bf16, tag="h1t")
                    nc.scalar.activation(h1t[:], po[:], AF.Relu)
                    nc.sync.dma_start(
                        ag_in[t][c0:c0 + MCH].rearrange("(j p) d -> p j d",
                                                        p=128),
                        h1t[:])

            def dense2(t, agg, pool, psum):
                for m in range(NCHK):
                    c0 = m * MCH
                    at01, at2z = load_aT(agg, c0, pool)
                    po2 = psum.tile([D, MCH], f32, tag="po2", space="PSUM")
                    nc.tensor.matmul(po2[:], wt["w2s"][:], at01[:],
                                     start=True, stop=False)
                    nc.tensor.matmul(po2[:], wt["w2r2"][:], at2z[:],
                                     start=False, stop=True)
                    h2t = pool.tile([D, MCH], f32, tag="h2t")
                    nc.scalar.activation(h2t[:], po2[:], AF.Relu)
                    nc.sync.dma_start(h2T[t][:, c0:c0 + MCH], h2t[:])

            with (
                tc.tile_pool(name="work", bufs=2) as pool,
                tc.tile_pool(name="ps", bufs=2, space="PSUM") as psum,
            ):
                for t in range(T):
                    # collectives cannot read IO tensors: stage via SBUF
                    xst = pool.tile([128, NSP // 128, D], bf16, tag="xst")
                    nc.sync.dma_start(
                        xst[:], xsh_d[t].rearrange("(j p) d -> p j d", p=128))
                    nc.sync.dma_start(
                        xin[t][:].rearrange("(j p) d -> p j d", p=128), xst[:])
                for t in range(T):
                    nc.gpsimd.collective_compute(
                        "AllGather", ALU.bypass,
                        replica_groups=[list(range(NC))],
                        ins=[xin[t][:]], outs=[featf[t][:]])
                for t in range(T):
                    agg = agg_d[t % 2]
                    edge_phase(t, featf[t], agg, pool)
                    dense1(t, agg, pool, psum)
                    nc.gpsimd.collective_compute(
                        "AllGather", ALU.bypass,
                        replica_groups=[list(range(NC))],
                        ins=[ag_in[t][:]], outs=[h1f[t][:]])
                for t in range(T):
                    agg = agg_d[t % 2]
                    edge_phase(t, h1f[t], agg, pool)
                    dense2(t, agg, pool, psum)

            # ---- GRU + MLP over feature-major chunks
            with (
                tc.tile_pool(name="gw", bufs=2) as gpool,
                tc.tile_pool(name="gp", bufs=1, space="PSUM") as gps,
            ):
                lrow = gpool.tile([1, NSP], f32, tag="lrow")
                for m in range(NCHK):
                    cols = slice(m * MCH, (m + 1) * MCH)
                    hA = gpool.tile([D, MCH], f32, tag="hA")
                    hB = gpool.tile([D, MCH], f32, tag="hB")
                    nc.vector.memset(hA[:], 0.0)
                    for t in range(T):
                        hin = hA if t % 2 == 0 else hB
                        hout = hB if t % 2 == 0 else hA
                        xT = gpool.tile([D, MCH], f32, tag="xT")
                        nc.sync.dma_start(xT[:], h2T[t][:, cols])
                        ps_r = gps.tile([D, MCH], f32, tag="ps_r", space="PSUM")
                        nc.tensor.matmul(ps_r[:], wt["wihrz"][:, 0:D], xT[:],
                                         start=True, stop=False)
                        nc.tensor.matmul(ps_r[:], wt["whhrz"][:, 0:D], hin[:],
                                         start=False, stop=True)
                        ps_z = gps.tile([D, MCH], f32, tag="ps_z", space="PSUM")
                        nc.tensor.matmul(ps_z[:], wt["wihrz"][:, D:2 * D],
                                         xT[:], start=True, stop=False)
                        nc.tensor.matmul(ps_z[:], wt["whhrz"][:, D:2 * D],
                                         hin[:], start=False, stop=True)
                        ps_n = gps.tile([D, MCH], f32, tag="ps_n", space="PSUM")
                        nc.tensor.matmul(ps_n[:], wt["wihn"][:], xT[:],
                                         start=True, stop=True)
                        ps_h = gps.tile([D, MCH], f32, tag="ps_h", space="PSUM")
                        nc.tensor.matmul(ps_h[:], wt["whhn"][:], hin[:],
                                         start=True, stop=True)
                        r_sb = gpool.tile([D, MCH], f32, tag="r_sb")
                        nc.scalar.activation(r_sb[:], ps_r[:], AF.Sigmoid,
                                             bias=wt["gbr"][:])
                        z_sb = gpool.tile([D, MCH], f32, tag="z_sb")
                        nc.scalar.activation(z_sb[:], ps_z[:], AF.Sigmoid,
                                             bias=wt["gbz"][:])
                        hn = gpool.tile([D, MCH], f32, tag="hn")
                        nc.scalar.activation(hn[:], ps_h[:], AF.Identity,
                                             bias=wt["gbhn"][:])
                        nc.vector.tensor_tensor(out=hn[:], in0=r_sb[:],
                                                in1=hn[:], op=ALU.mult)
                        nc.vector.tensor_tensor(out=hn[:], in0=ps_n[:],
                                                in1=hn[:], op=ALU.add)
                        nt = gpool.tile([D, MCH], f32, tag="nt")
                        nc.scalar.activation(nt[:], hn[:], AF.Tanh,
                                             bias=wt["gbin"][:])
                        nc.vector.tensor_tensor(out=hout[:], in0=hin[:],
                                                in1=nt[:], op=ALU.subtract)
                        nc.vector.tensor_tensor(out=hout[:], in0=z_sb[:],
                                                in1=hout[:], op=ALU.mult)
                        nc.vector.tensor_tensor(out=hout[:], in0=nt[:],
                                                in1=hout[:], op=ALU.add)
                    hlast = hA if T % 2 == 0 else hB
                    ps_f = gps.tile([D, MCH], f32, tag="ps_f", space="PSUM")
                    nc.tensor.matmul(ps_f[:], wt["wc1"][:], hlast[:],
                                     start=True, stop=True)
                    zf = gpool.tile([D, MCH], f32, tag="zf")
                    nc.scalar.activation(zf[:], ps_f[:], AF.Relu,
                                         bias=wt["bc1c"][:])
                    ps_l = gps.tile([1, MCH], f32, tag="ps_l", space="PSUM")
                    nc.tensor.matmul(ps_l[:], wt["wc2"][:], zf[:],
                                     start=True, stop=True)
                    nc.scalar.activation(lrow[:, cols], ps_l[:], AF.Identity,
                                         bias=wt["bc2c"][:])
                nc.sync.dma_start(out_d[:], lrow[:, 0:NS])

    nc.compile()
    return nc


# --------------------------------------------------------------------------
# fast PJRT runner: pre-place sharded inputs with device_put so the jitted
# shard_map call doesn't re-transfer unsharded numpy args through the relay
# --------------------------------------------------------------------------

def _early_upload(in_maps):
    """Concat per-core inputs and start async sharded device transfers.

    Called BEFORE build_program so the ~5s relay transfer overlaps the
    ~13s bass build. Returns {name: committed jax.Array}.
    """
    import jax
    from jax.sharding import Mesh, PartitionSpec, NamedSharding

    devices = jax.devices()[:NC]
    mesh = Mesh(np.asarray(devices), ("core",))
    sh = NamedSharding(mesh, PartitionSpec("core"))
    dev_map = {}
    for name in in_maps[0]:
        cat = np.concatenate([np.asarray(m[name]) for m in in_maps], axis=0)
        dev_map[name] = jax.device_put(cat, sh)
    return dev_map


def _install_fast_pjrt_runner(dev_map):
    import jax
    from jax.sharding import Mesh, PartitionSpec, NamedSharding
    from jax.experimental.shard_map import shard_map
    from concourse import bass2jax, mybir
    from concourse.bass2jax import (_bass_exec_p, install_neuronx_cc_hook,
                                    partition_id_tensor)

    def run_fast(nc, in_maps, n_cores):
        install_neuronx_cc_hook()
        partition_name = (nc.partition_id_tensor.name
                          if nc.partition_id_tensor else None)
        in_names, out_names, out_avals, zero_outs = [], [], [], []
        for alloc in nc.m.functions[0].allocations:
            if not isinstance(alloc, mybir.MemoryLocationSet):
                continue
            name = alloc.memorylocations[0].name
            if alloc.kind == "ExternalInput":
                if name != partition_name:
                    in_names.append(name)
            elif alloc.kind == "ExternalOutput":
                out_names.append(name)
                shape = tuple(alloc.tensor_shape)
                dtype = mybir.dt.np(alloc.dtype)
                out_avals.append(jax.core.ShapedArray(shape, dtype))
                zero_outs.append(np.zeros(shape, dtype))
        n_params = len(in_names)
        in_names.extend(out_names)
        if partition_name is not None:
            in_names.append(partition_name)

        def _body(*args):
            operands = list(args)
            if partition_name is not None:
                operands.append(partition_id_tensor())
            outs = _bass_exec_p.bind(
                *operands, out_avals=tuple(out_avals),
                in_names=tuple(in_names), out_names=tuple(out_names),
                lowering_input_output_aliases=(),
                sim_require_finite=True, sim_require_nnan=True, nc=nc)
            return tuple(outs)

        devices = jax.devices()[:n_cores]
        mesh = Mesh(np.asarray(devices), ("core",))
        n_outs = len(out_avals)
        in_specs = (PartitionSpec("core"),) * (n_params + n_outs)
        out_specs = (PartitionSpec("core"),) * len(out_names)
        sharded = jax.jit(
            shard_map(_body, mesh=mesh, in_specs=in_specs,
                      out_specs=out_specs, check_rep=False),
            keep_unused=True)
        concat_in = [
            (None if dev_map.get(name) is not None else
             np.concatenate([np.asarray(m[name]) for m in in_maps], axis=0))
            for name in in_names[:n_params]]
        concat_zeros = [
            np.zeros((n_cores * z.shape[0], *z.shape[1:]), z.dtype)
            for z in zero_outs]
        import time as _time
        _t0 = _time.monotonic()
        sh = NamedSharding(mesh, PartitionSpec("core"))
        dev_in = [dev_map.get(n) if dev_map.get(n) is not None
                  else jax.device_put(concat_in[i], sh)
                  for i, n in enumerate(in_names[:n_params])]
        dev_zero = [jax.device_put(z, sh) for z in concat_zeros]
        for a in dev_in:
            a.block_until_ready()
        _t1 = _time.monotonic()
        out_arrs = sharded(*dev_in, *dev_zero)
        for o in out_arrs:
            o.block_until_ready()
        _t2 = _time.monotonic()
        print(f"[kernel] upload-wait {_t1-_t0:.1f}s  exec {_t2-_t1:.1f}s",
              file=sys.stderr, flush=True)
        return [
            {name: np.asarray(out_arrs[i]).reshape(
                n_cores, *out_avals[i].shape)[c]
             for i, name in enumerate(out_names)}
            for c in range(n_cores)
        ]

    bass2jax.run_bass_via_pjrt = run_fast


# --------------------------------------------------------------------------
# entry point
# --------------------------------------------------------------------------

def kernel(**inputs):
    import time

    per_core, tiles = preprocess(inputs["src"], inputs["dst"], inputs["ew"])
    wts = make_weights(
        inputs["W1"], inputs["b1"], inputs["W2"], inputs["b2"],
        inputs["Wih"], inputs["Whh"], inputs["bih"], inputs["bhh"],
        inputs["Wc1"], inputs["bc1"], inputs["Wc2"], inputs["bc2"])
    shards = make_feat_shards(inputs["feat"])

    in_maps = []
    for cc in range(NC):
        m = dict(per_core[cc])
        m.update(wts)
        m["xsh"] = shards[cc]
        in_maps.append(m)

    # kick off async sharded upload, then overlap it with the bass build
    dev_map = _early_upload(in_maps)
    nc = build_program(tiles)
    _install_fast_pjrt_runner(dev_map)
    from concourse.bass_utils import run_bass_kernel_spmd
    kwargs = {}
    if TRACE:
        kwargs = dict(trace=True, trace_cores=list(range(NC)))
    t0 = time.monotonic()
    try:
        res = run_bass_kernel_spmd(nc, in_maps, list(range(NC)), **kwargs)
    except (ImportError, ModuleNotFoundError):
        # NTFF profiling hook unavailable in this environment
        res = run_bass_kernel_spmd(nc, in_maps, list(range(NC)))
    wall_ns = (time.monotonic() - t0) * 1e9
    global LAST_EXEC_NS
    LAST_EXEC_NS = res.exec_time_ns if res.exec_time_ns else int(wall_ns)
    out = np.concatenate(
        [np.asarray(res.results[cc]["out"]).reshape(NS) for cc in range(NC)])
    return out.astype(np.float32)


if __name__ == "__main__":
    pass
